# revision 17
# baseline (speedup 1.0000x reference)
"""AggregatedAttention TRN2 kernel: B=16 sharded 2-per-core over 8 NeuronCores.

Layout: channel-major [C=128 partitions, N=3136 free]. Local 3x3 window
attention via 9 shifted DVE products + PE indicator-matmul head-sums; pooled
branch via block-diagonal matmuls; fused softmax over [9 local | 49 pool]
without max-subtraction (logits bounded ~19). All f32 (optional fp32r matmuls).
"""

import os
from contextlib import ExitStack

import numpy as np

import concourse.bass as bass
import concourse.tile as tile
from concourse import bacc, mybir
from concourse._compat import with_exitstack
from concourse.bass_utils import run_bass_kernel_spmd

F32 = mybir.dt.float32

B, N, C = 16, 3136, 128
H = W = 56
HP = WP = 58
NP = HP * WP  # 3364
NH, HD, LL, PL = 4, 32, 9, 49
SR = 8
PH = PW = 7
CH = 448
NCH = 7
NCORES = 8
BPC = 2
NEGM = -60.0
USE_F32R = os.environ.get("KERNEL_F32R", "1") == "1"


def _softplus(x):
    return np.log1p(np.exp(-np.abs(x))) + np.maximum(x, 0.0)


def _valid_counts():
    """cnt[n] = number of in-image neighbors (ones-unfold column sums)."""
    ones = np.ones((H, W), np.float32)
    pad = np.pad(ones, 1)
    cnt = np.zeros((H, W), np.float32)
    for i in range(3):
        for j in range(3):
            cnt += pad[i:i + H, j:j + W]
    return cnt.reshape(-1)


def host_prep(inputs):
    """Build per-core input maps (numpy) + list of (name, shape) for DRAM decl."""
    x = np.asarray(inputs["x"], np.float32)
    rct = np.asarray(inputs["relative_coords_table"], np.float32)
    q_w = np.asarray(inputs["q_w"], np.float32)
    q_b = np.asarray(inputs["q_b"], np.float32)
    kv_w = np.asarray(inputs["kv_w"], np.float32)
    kv_b = np.asarray(inputs["kv_b"], np.float32)
    temperature = np.asarray(inputs["temperature"], np.float32)
    query_embedding = np.asarray(inputs["query_embedding"], np.float32)
    sr_w = np.asarray(inputs["sr_w"], np.float32)
    sr_b = np.asarray(inputs["sr_b"], np.float32)
    norm_g = np.asarray(inputs["norm_g"], np.float32)
    norm_b = np.asarray(inputs["norm_b"], np.float32)
    cpb1_w = np.asarray(inputs["cpb1_w"], np.float32)
    cpb1_b = np.asarray(inputs["cpb1_b"], np.float32)
    cpb2_w = np.asarray(inputs["cpb2_w"], np.float32)
    cpb2_b = np.asarray(inputs["cpb2_b"], np.float32)
    rel_bias_local = np.asarray(inputs["rel_bias_local"], np.float32)
    learnable_tokens = np.asarray(inputs["learnable_tokens"], np.float32)
    learnable_bias = np.asarray(inputs["learnable_bias"], np.float32)
    proj_w = np.asarray(inputs["proj_w"], np.float32)
    proj_b = np.asarray(inputs["proj_b"], np.float32)
    rpi = np.asarray(inputs["relative_pos_index"]).astype(np.int64)

    # ---- CPB MLP + gather -> pool_bias [NH, N, PL] (batch independent) ----
    cpb = np.maximum(rct @ cpb1_w + cpb1_b, 0.0) @ cpb2_w + cpb2_b  # [TBL, NH]
    pb = cpb.T[:, rpi].reshape(NH, N, PL)  # [4, N, 49]
    PB_A = np.zeros((128, N), np.float32)
    PB_B = np.zeros((128, N), np.float32)
    for h in range(2):
        PB_A[64 * h:64 * h + 49] = pb[h].T
        PB_B[64 * h:64 * h + 49] = pb[2 + h].T

    sp = _softplus(temperature).reshape(NH)  # [4]
    seq = np.log(_valid_counts() + PL)  # [N]

    sseq_b = np.zeros((128, N), np.float32)
    for h in range(NH):
        sseq_b[32 * h:32 * h + 32] = sp[h] * seq[None, :]

    emb128 = query_embedding.reshape(NH, HD).reshape(-1, 1).astype(np.float32)

    # MB36: rel_bias if valid else NEGM, [36, N], row 4l+h
    MB36 = np.full((36, N), NEGM, np.float32)
    rr, cc = np.divmod(np.arange(N), W)
    for i in range(3):
        for j in range(3):
            l = 3 * i + j
            valid = (rr + i - 1 >= 0) & (rr + i - 1 < H) & \
                    (cc + j - 1 >= 0) & (cc + j - 1 < W)
            for h in range(NH):
                MB36[4 * l + h] = np.where(valid, rel_bias_local[h, l], NEGM)

    b36 = np.zeros((36, 1), np.float32)
    LT_bd = np.zeros((128, 36), np.float32)
    for l in range(LL):
        for h in range(NH):
            b36[4 * l + h, 0] = learnable_bias[h, 0, l]
            LT_bd[32 * h:32 * h + 32, 4 * l + h] = learnable_tokens[h, :, l]

    L36IND = np.zeros((LL, 128, 36), np.float32)
    SB = np.zeros((LL, 36, 128), np.float32)
    for l in range(LL):
        for h in range(NH):
            L36IND[l, 32 * h:32 * h + 32, 4 * l + h] = 1.0
            SB[l, 4 * l + h, 32 * h:32 * h + 32] = 1.0

    IND28S = np.zeros((NCH, 128, 28), np.float32)
    D36_28 = np.zeros((NCH, 36, 28), np.float32)
    DP28A = np.zeros((NCH, 128, 28), np.float32)
    DP28B = np.zeros((NCH, 128, 28), np.float32)
    B36S = np.zeros((NCH, 28, 36), np.float32)
    BAS = np.zeros((NCH, 28, 128), np.float32)
    for c in range(NCH):
        for h in range(NH):
            IND28S[c, 32 * h:32 * h + 32, 4 * c + h] = 1.0
            BAS[c, 4 * c + h, 32 * h:32 * h + 32] = 1.0
            for l in range(LL):
                D36_28[c, 4 * l + h, 4 * c + h] = 1.0
                B36S[c, 4 * c + h, 4 * l + h] = 1.0
        for h in range(2):
            DP28A[c, 64 * h:64 * h + 49, 4 * c + h] = 1.0
            DP28B[c, 64 * h:64 * h + 49, 4 * c + (2 + h)] = 1.0

    B4to128 = np.zeros((4, 128), np.float32)
    ones4 = np.zeros((128, 4), np.float32)
    for h in range(NH):
        B4to128[h, 32 * h:32 * h + 32] = 1.0
        ones4[32 * h:32 * h + 32, h] = 1.0

    # LN folded into pool kv projection
    WKg = kv_w * norm_g[:, None]  # [128, 256]
    m = norm_g @ kv_w  # [256]
    ckb = norm_b @ kv_w + kv_b  # [256]

    common = {
        "PB_A": PB_A,
        "PB_B": PB_B,
        "sseq_b": sseq_b,
        "emb128": emb128,
        "MB36": MB36,
        "b36": b36,
        "LT_bd": LT_bd,
        "L36IND": L36IND.transpose(1, 0, 2).reshape(128, LL * 36),
        "SBm": SB.transpose(1, 0, 2).reshape(36, LL * 128),
        "IND28S": IND28S.transpose(1, 0, 2).reshape(128, NCH * 28),
        "D36_28": D36_28.transpose(1, 0, 2).reshape(36, NCH * 28),
        "DP28A": DP28A.transpose(1, 0, 2).reshape(128, NCH * 28),
        "DP28B": DP28B.transpose(1, 0, 2).reshape(128, NCH * 28),
        "B36S": B36S.transpose(1, 0, 2).reshape(28, NCH * 36),
        "BAS": BAS.transpose(1, 0, 2).reshape(28, NCH * 128),
        "B4to128": B4to128,
        "ones4": ones4,
        "I36": np.eye(36, dtype=np.float32),
        "I98": np.eye(128, dtype=np.float32),
        "ones49": np.ones((1, PL), np.float32),
        "ones128c": np.ones((128, 1), np.float32),
        "ones1to128": np.ones((1, 128), np.float32),
        "WQ": q_w,
        "WK": np.ascontiguousarray(kv_w[:, :128]),
        "WV": np.ascontiguousarray(kv_w[:, 128:]),
        "WSRT": np.ascontiguousarray(sr_w.T),
        "WP": proj_w,
        "WKg_k": np.ascontiguousarray(WKg[:, :128]),
        "WVg_v": np.ascontiguousarray(WKg[:, 128:]),
        "negm_k": np.ascontiguousarray(-m[None, :128]),
        "negm_v": np.ascontiguousarray(-m[None, 128:]),
        "ckb_k": np.ascontiguousarray(ckb[None, :128]),
        "ckb_v": np.ascontiguousarray(ckb[None, 128:]),
        "bq_r": q_b[None, :],
        "bk_r": np.ascontiguousarray(kv_b[None, :128]),
        "bv_r": np.ascontiguousarray(kv_b[None, 128:]),
        "bsr_col": sr_b[:, None],
        "bq_col": q_b[:, None],
        "bk_col": np.ascontiguousarray(kv_b[:128, None]),
        "bv_col": np.ascontiguousarray(kv_b[128:, None]),
        "bp_col": proj_b[:, None],
        "epsLN": np.full((1, 1), 64.0 * 64.0 * 1e-5, np.float32),
    }
    common = {k: np.ascontiguousarray(v, dtype=np.float32) for k, v in common.items()}

    xT = np.ascontiguousarray(x.transpose(0, 2, 1))  # [16, 128, N]
    in_maps = []
    for core in range(NCORES):
        m_ = dict(common)
        m_["xT"] = np.ascontiguousarray(
            xT[BPC * core:BPC * core + BPC].reshape(BPC * 128, N))
        in_maps.append(m_)
    return in_maps


IN_SHAPES = {
    "xT": (BPC * 128, N), "PB_A": (128, N), "PB_B": (128, N),
    "sseq_b": (128, N),
    "emb128": (128, 1), "MB36": (36, N), "b36": (36, 1), "LT_bd": (128, 36),
    "L36IND": (128, LL * 36), "SBm": (36, LL * 128), "IND28S": (128, NCH * 28),
    "D36_28": (36, NCH * 28), "DP28A": (128, NCH * 28), "DP28B": (128, NCH * 28),
    "B36S": (28, NCH * 36), "BAS": (28, NCH * 128), "B4to128": (4, 128),
    "ones4": (128, 4), "I36": (36, 36), "I98": (128, 128),
    "ones49": (1, PL), "ones128c": (128, 1), "ones1to128": (1, 128),
    "WQ": (128, 128), "WK": (128, 128), "WV": (128, 128), "WSRT": (128, 128),
    "WP": (128, 128), "WKg_k": (128, 128), "WVg_v": (128, 128),
    "negm_k": (1, 128), "negm_v": (1, 128), "ckb_k": (1, 128),
    "ckb_v": (1, 128), "bq_r": (1, 128), "bk_r": (1, 128), "bv_r": (1, 128),
    "bsr_col": (128, 1), "bp_col": (128, 1), "epsLN": (1, 1),
    "bq_col": (128, 1), "bk_col": (128, 1), "bv_col": (128, 1),
}


def _r(ap):
    """Optionally bitcast an AP to float32r for fast PE streaming."""
    return ap.bitcast(mybir.dt.float32r) if USE_F32R else ap


@with_exitstack
def build_tile_kernel(ctx: ExitStack, tc: tile.TileContext, outs, ins):
    nc = tc.nc
    mm = nc.tensor.matmul
    Act = mybir.ActivationFunctionType
    Alu = mybir.AluOpType

    outT = outs["outT"]  # dram [256, N]

    # ---- load constants into SBUF (bufs=1 pools, persist whole kernel) ----
    cpool = ctx.enter_context(tc.tile_pool(name="consts", bufs=1))
    cs = {}
    for name in IN_SHAPES:
        if name == "xT":
            continue
        t = cpool.tile(list(IN_SHAPES[name]), F32, tag=name)
        nc.sync.dma_start(t[:], ins[name])
        cs[name] = t

    def c3(name, i, cols):
        """slice i of a free-dim-stacked [rows, k*cols] const -> [rows, cols] AP"""
        return cs[name][:, i * cols:(i + 1) * cols]

    sb = ctx.enter_context(tc.tile_pool(name="sb", bufs=1))
    sbch = ctx.enter_context(tc.tile_pool(name="sbch", bufs=2))
    sbch1 = ctx.enter_context(tc.tile_pool(name="sbch1", bufs=1))
    psA = ctx.enter_context(tc.tile_pool(name="psA", bufs=2, space="PSUM"))
    psB = ctx.enter_context(tc.tile_pool(name="psB", bufs=2, space="PSUM"))
    psL = ctx.enter_context(tc.tile_pool(name="psL", bufs=2, space="PSUM"))
    psAcc = ctx.enter_context(tc.tile_pool(name="psAcc", bufs=1, space="PSUM"))
    psP1 = ctx.enter_context(tc.tile_pool(name="psP1", bufs=1, space="PSUM"))

    # persistent per-sample SBUF tensors (tags reused across samples)
    xTs = sb.tile([128, N], F32, tag="xTs")
    qbig = sb.tile([128, N], F32, tag="qbig")
    kpad = sb.tile([128, NP], F32, tag="kpad")
    vpad = sb.tile([128, NP], F32, tag="vpad")
    e36 = sb.tile([36, N], F32, tag="e36")
    sfin = sb.tile([36, N], F32, tag="sfin")
    expPA = sb.tile([128, N], F32, tag="expPA")
    expPB = sb.tile([128, N], F32, tag="expPB")
    x_pooled = sb.tile([128, PL], F32, tag="x_pooled")
    KPbdA = sb.tile([128, 128], F32, tag="KPbdA")
    KPbdB = sb.tile([128, 128], F32, tag="KPbdB")
    VPbdA = sb.tile([128, 128], F32, tag="VPbdA")
    VPbdB = sb.tile([128, 128], F32, tag="VPbdB")
    pk28 = sb.tile([28, 3 * CH], F32, tag="pk28")
    aQ28 = pk28[:, 0:CH]
    aK28 = pk28[:, CH:2 * CH]
    invD28 = pk28[:, 2 * CH:3 * CH]

    for t in (kpad, vpad, KPbdA, KPbdB, VPbdA, VPbdB):
        nc.vector.memset(t[:], 0.0)

    kpad3 = kpad[:].rearrange("p (a b) -> p a b", a=HP)
    vpad3 = vpad[:].rearrange("p (a b) -> p a b", a=HP)

    def chunk(t, c, rows=slice(None)):
        return t[rows, c * CH:(c + 1) * CH]

    def chv(ap):  # [*, 448] -> [*, 8, 56]
        return ap.rearrange("p (a b) -> p a b", a=8)

    def proj_psum(pool, wname, c, tag):
        p = pool.tile([128, CH], F32, tag=tag)
        mm(p[:], _r(cs[wname][:]), _r(chunk(xTs, c)), start=True, stop=True)
        return p


    for s in range(BPC):
        nc.sync.dma_start(xTs[:], ins["xT"][128 * s:128 * s + 128, :])

        # ---- PH-A: first-pass projections: squares for norms, v, sr ----
        ssqk = psAcc.tile([64, CH], F32, tag="ssq")
        sumsqQ = ssqk[0:28, :]
        sumsqK = ssqk[32:60, :]
        for c in range(NCH):
            qp = proj_psum(psA, "WQ", c, "prj")
            q2c = sbch.tile([128, CH], F32, tag="sq2")
            nc.scalar.activation(q2c[:], qp[:], Act.Square, bias=cs["bq_col"][:, 0:1])
            mm(sumsqQ, _r(c3("IND28S", c, 28)), _r(q2c[:]),
               start=(c == 0), stop=(c == NCH - 1), skip_group_check=True)

            kp = proj_psum(psA, "WK", c, "prj")
            k2c = sbch.tile([128, CH], F32, tag="sq2")
            nc.scalar.activation(k2c[:], kp[:], Act.Square, bias=cs["bk_col"][:, 0:1])
            mm(sumsqK, _r(c3("IND28S", c, 28)), _r(k2c[:]),
               start=(c == 0), stop=(c == NCH - 1), skip_group_check=True)

            vp = proj_psum(psA, "WV", c, "prj")
            nc.scalar.activation(vpad3[:, c * 8 + 1:c * 8 + 9, 1:57], chv(vp[:]),
                                 Act.Identity, bias=cs["bv_col"][:, 0:1])

            srp = proj_psum(psA, "WSRT", c, "prj")
            xsrc = sbch.tile([128, CH], F32, tag="xsr")
            nc.scalar.activation(xsrc[:], srp[:], Act.Gelu, bias=cs["bsr_col"][:, 0:1])
            st1 = sbch1.tile([128, 56], F32, tag="st1")
            nc.vector.tensor_reduce(
                st1[:], xsrc[:].rearrange("p (r pc cc) -> p r pc cc", r=8, pc=7),
                axis=mybir.AxisListType.X, op=Alu.add)
            nc.vector.tensor_reduce(
                x_pooled[:, c * 7:c * 7 + 7],
                st1[:].rearrange("p (r pc) -> p pc r", r=8),
                axis=mybir.AxisListType.X, op=Alu.add)

        # ---- PH-B: alpha = rsqrt(sumsq) via exp(-0.5*log) ----
        nc.scalar.activation(aQ28, sumsqQ, Act.Ln)
        nc.scalar.activation(aQ28, aQ28, Act.Exp, scale=-0.5)
        nc.scalar.activation(aK28, sumsqK, Act.Ln)
        nc.scalar.activation(aK28, aK28, Act.Exp, scale=-0.5)

        # ---- PH-C: rebuild q/k, normalize; qhat, qbig, khat->kpad ----
        for c in range(NCH):
            qp = proj_psum(psA, "WQ", c, "prj")
            ab = psB.tile([128, CH], F32, tag="bc")
            mm(ab[:], _r(c3("BAS", c, 128)), _r(aQ28), start=True, stop=True)
            qsb = sbch1.tile([128, CH], F32, tag="qsb")
            nc.scalar.activation(qsb[:], qp[:], Act.Identity, bias=cs["bq_col"][:, 0:1])
            qhc = sbch.tile([128, CH], F32, tag="qh")
            nc.vector.tensor_mul(qhc[:], qsb[:], ab[:])
            nc.vector.scalar_tensor_tensor(
                chunk(qbig, c), qhc[:], cs["emb128"][:, 0:1],
                chunk(cs["sseq_b"], c), op0=Alu.add, op1=Alu.mult)
            t36 = psL.tile([36, CH], F32, tag="l36")
            mm(t36[:], _r(cs["LT_bd"][:]), _r(qhc[:]), start=True, stop=True)
            nc.scalar.activation(chunk(sfin, c), t36[:], Act.Identity,
                                 bias=cs["b36"][:, 0:1])

            kp = proj_psum(psA, "WK", c, "prj")
            abk = psB.tile([128, CH], F32, tag="bc")
            mm(abk[:], _r(c3("BAS", c, 128)), _r(aK28), start=True, stop=True)
            ksb = sbch1.tile([128, CH], F32, tag="ksb")
            nc.scalar.activation(ksb[:], kp[:], Act.Identity, bias=cs["bk_col"][:, 0:1])
            nc.vector.tensor_tensor(
                kpad3[:, c * 8 + 1:c * 8 + 9, 1:57], ksb[:].rearrange(
                    "p (a b) -> p a b", a=8), abk[:].rearrange(
                    "p (a b) -> p a b", a=8), op=Alu.mult)

        # ---- PH-D: pooled branch ----
        SS = psP1.tile([1, 2 * PL], F32, tag="pl")
        S1 = SS[:, 0:PL]
        S2 = SS[:, PL:2 * PL]
        mm(S1, cs["ones128c"][:], x_pooled[:], start=True, stop=True)
        xp2 = sbch1.tile([128, PL], F32, tag="pl128")
        nc.scalar.square(xp2[:], x_pooled[:])
        mm(S2, cs["ones128c"][:], xp2[:], start=True, stop=True)
        mu2 = sbch1.tile([1, PL], F32, tag="pl1a")
        nc.scalar.activation(mu2[:], S1, Act.Square, scale=1.0 / 128)
        S2s = sbch1.tile([1, PL], F32, tag="pl1b")
        nc.scalar.mul(S2s[:], S2, 1.0 / 128)
        Ev = sbch1.tile([1, PL], F32, tag="pl1c")
        nc.vector.tensor_sub(Ev[:], S2s[:], mu2[:])
        lgE = sbch1.tile([1, PL], F32, tag="pl1d")
        nc.scalar.activation(lgE[:], Ev[:], Act.Ln, bias=cs["epsLN"][:, 0:1])
        invsig = sbch1.tile([1, PL], F32, tag="pl1e")
        nc.scalar.activation(invsig[:], lgE[:], Act.Exp, scale=-0.5)
        muinvs = sbch1.tile([1, PL], F32, tag="pl1f")
        nc.vector.scalar_tensor_tensor(
            muinvs[:], S1, 1.0 / 128, invsig[:], op0=Alu.mult, op1=Alu.mult)
        isb = psB.tile([128, PL], F32, tag="bc")
        mm(isb[:], cs["ones1to128"][:], invsig[:], start=True, stop=True)
        xpn = sbch1.tile([128, PL], F32, tag="pl128b")
        nc.vector.tensor_mul(xpn[:], x_pooled[:], isb[:])

        kpl = psA.tile([128, PL], F32, tag="prj")
        mm(kpl[:], _r(cs["WKg_k"][:]), _r(xpn[:]), start=True, stop=False)
        mm(kpl[:], cs["negm_k"][:], muinvs[:], start=False, stop=False)
        mm(kpl[:], cs["ckb_k"][:], cs["ones49"][:], start=False, stop=True)
        kp2s = sbch1.tile([128, PL], F32, tag="pl128")
        nc.scalar.square(kp2s[:], kpl[:])
        ssP = psAcc.tile([4, PL], F32, tag="ssq")
        mm(ssP[:], cs["ones4"][:], kp2s[:], start=True, stop=True)
        lgP = sbch1.tile([4, PL], F32, tag="pl4a")
        nc.scalar.activation(lgP[:], ssP[:], Act.Ln)
        aP = sbch1.tile([4, PL], F32, tag="pl4b")
        nc.scalar.activation(aP[:], lgP[:], Act.Exp, scale=-0.5)
        aPb = psB.tile([128, PL], F32, tag="bc")
        mm(aPb[:], cs["B4to128"][:], aP[:], start=True, stop=True)
        kplsb = sbch1.tile([128, PL], F32, tag="pl128c")
        nc.scalar.copy(kplsb[:], kpl[:])
        khp = sbch1.tile([128, PL], F32, tag="pl128b")
        nc.vector.tensor_mul(khp[:], kplsb[:], aPb[:])
        for h in range(2):
            nc.vector.tensor_copy(
                KPbdA[32 * h:32 * h + 32, 64 * h:64 * h + 49],
                khp[32 * h:32 * h + 32, :])
            hh = 2 + h
            nc.vector.tensor_copy(
                KPbdB[32 * hh:32 * hh + 32, 64 * h:64 * h + 49],
                khp[32 * hh:32 * hh + 32, :])

        vplT = psA.tile([PL, 128], F32, tag="prj")
        mm(vplT[:], _r(xpn[:]), _r(cs["WVg_v"][:]), start=True, stop=False)
        mm(vplT[:], muinvs[:], cs["negm_v"][:], start=False, stop=False)
        mm(vplT[:], cs["ones49"][:], cs["ckb_v"][:], start=False, stop=True)
        vpT = sbch1.tile([PL, 128], F32, tag="plT")
        nc.scalar.copy(vpT[:], vplT[:])
        for h in range(2):
            nc.vector.tensor_copy(
                VPbdA[64 * h:64 * h + 49, 32 * h:32 * h + 32],
                vpT[:, 32 * h:32 * h + 32])
            hh = 2 + h
            nc.vector.tensor_copy(
                VPbdB[64 * h:64 * h + 49, 32 * hh:32 * hh + 32],
                vpT[:, 32 * hh:32 * hh + 32])

        # ---- PH-E: pooled logits + exp ----
        for c in range(NCH):
            ppA = psB.tile([128, CH], F32, tag="bc")
            mm(ppA[:], _r(KPbdA[:]), _r(chunk(qbig, c)), start=True, stop=False)
            mm(ppA[:], _r(cs["I98"][:]), _r(chunk(cs["PB_A"], c)),
               start=False, stop=True)
            nc.scalar.activation(chunk(expPA, c), ppA[:], Act.Exp)
            ppB = psB.tile([128, CH], F32, tag="bc")
            mm(ppB[:], _r(KPbdB[:]), _r(chunk(qbig, c)), start=True, stop=False)
            mm(ppB[:], _r(cs["I98"][:]), _r(chunk(cs["PB_B"], c)),
               start=False, stop=True)
            nc.scalar.activation(chunk(expPB, c), ppB[:], Act.Exp)

        # ---- PH-F: local logits (9 shifted products), exp, t36 ----
        for c in range(NCH):
            L36 = psL.tile([36, CH], F32, tag="l36")
            for l in range(LL):
                i, j = divmod(l, 3)
                prod = sbch.tile([128, CH], F32, tag="prod")
                nc.vector.tensor_tensor(
                    chv(prod[:]), chv(chunk(qbig, c)),
                    kpad3[:, c * 8 + i:c * 8 + i + 8, j:j + 56], op=Alu.mult)
                mm(L36[:], _r(c3("L36IND", l, 36)), _r(prod[:]),
                   start=(l == 0), stop=False)
            mm(L36[:], _r(cs["I36"][:]), _r(chunk(cs["MB36"], c)),
               start=False, stop=True)
            nc.scalar.activation(chunk(e36, c), L36[:], Act.Exp)

        # ---- PH-G: denominator + reciprocal ----
        D28 = psAcc.tile([28, CH], F32, tag="ssq")
        for c in range(NCH):
            mm(D28[:], _r(c3("D36_28", c, 28)), _r(chunk(e36, c)),
               start=(c == 0), stop=False, skip_group_check=True)
            mm(D28[:], _r(c3("DP28A", c, 28)), _r(chunk(expPA, c)),
               start=False, stop=False, skip_group_check=True)
            mm(D28[:], _r(c3("DP28B", c, 28)), _r(chunk(expPB, c)),
               start=False, stop=(c == NCH - 1), skip_group_check=True)
        nc.scalar.activation(invD28, D28[:], Act.Ln)
        nc.scalar.activation(invD28, invD28, Act.Exp, scale=-1.0)

        # ---- PH-H: s_final = e36 * invD_b + t36s ----
        for c in range(NCH):
            iDb = psL.tile([36, CH], F32, tag="l36")
            mm(iDb[:], _r(c3("B36S", c, 36)), _r(invD28), start=True, stop=True)
            tmp36 = sbch1.tile([36, CH], F32, tag="xpn")
            nc.vector.tensor_mul(tmp36[:], chunk(e36, c), iDb[:])
            nc.vector.tensor_add(chunk(sfin, c), chunk(sfin, c), tmp36[:])

        # ---- PH-I: x_pool, s*v products, proj, output ----
        for c in range(NCH):
            xpp = psA.tile([128, CH], F32, tag="prj")
            mm(xpp[:], _r(VPbdA[:]), _r(chunk(expPA, c)),
               start=True, stop=False)
            mm(xpp[:], _r(VPbdB[:]), _r(chunk(expPB, c)),
               start=False, stop=True)
            iD128 = psB.tile([128, CH], F32, tag="bc")
            mm(iD128[:], _r(c3("BAS", c, 128)), _r(invD28), start=True, stop=True)
            xppsb = sbch1.tile([128, CH], F32, tag="xppsb")
            nc.scalar.copy(xppsb[:], xpp[:])
            xpnc = sbch1.tile([128, CH], F32, tag="xpn")
            nc.vector.tensor_mul(xpnc[:], xppsb[:], iD128[:])

            projp = psA.tile([128, CH], F32, tag="prj")
            for l in range(LL):
                i, j = divmod(l, 3)
                sbc = psB.tile([128, CH], F32, tag="bc")
                mm(sbc[:], _r(c3("SBm", l, 128)), _r(chunk(sfin, c)),
                   start=True, stop=True)
                prod2 = sbch.tile([128, CH], F32, tag="prod")
                nc.vector.tensor_tensor(
                    chv(prod2[:]), vpad3[:, c * 8 + i:c * 8 + i + 8, j:j + 56],
                    chv(sbc[:]), op=Alu.mult)
                mm(projp[:], _r(cs["WP"][:]), _r(prod2[:]),
                   start=(l == 0), stop=False)
            mm(projp[:], _r(cs["WP"][:]), _r(xpnc[:]), start=False, stop=True)
            outc = sbch.tile([128, CH], F32, tag="outc")
            nc.scalar.activation(outc[:], projp[:], Act.Identity,
                                 bias=cs["bp_col"][:, 0:1])
            nc.sync.dma_start(
                outT[128 * s:128 * s + 128, c * CH:(c + 1) * CH], outc[:])


_CACHE = {}


def _get_program():
    if "nc" in _CACHE:
        return _CACHE["nc"]
    nc = bacc.Bacc("TRN2", target_bir_lowering=False, debug=False,
                   num_devices=NCORES)
    ins = {}
    for name, shp in IN_SHAPES.items():
        ins[name] = nc.dram_tensor(name, list(shp), F32,
                                   kind="ExternalInput").ap()
    outs = {"outT": nc.dram_tensor("outT", [BPC * 128, N], F32,
                                   kind="ExternalOutput").ap()}
    with tile.TileContext(nc) as tc:
        build_tile_kernel(tc, outs, ins)
    nc.compile()
    _CACHE["nc"] = nc
    return nc


def kernel(**inputs):
    in_maps = host_prep(inputs)
    nc = _get_program()
    res = run_bass_kernel_spmd(nc, in_maps, core_ids=list(range(NCORES)))
    outs = np.concatenate([r["outT"].reshape(BPC, 128, N)
                           for r in res.results], axis=0)  # [16, 128, N]
    return np.ascontiguousarray(outs.transpose(0, 2, 1))
